# revision 31
# baseline (speedup 1.0000x reference)
"""Trainium2 Bass kernel for a LoRA-MoE layer (gate top-2 softmax routing +
dense base linear + per-expert low-rank adapters), SPMD across 8 NeuronCores.

Math (per token t):
    logits = x @ gate_w.T                      # [E]
    top-2 softmax over logits -> dense w[E] (0 for non-selected)
    out = x @ base_w.T + base_b
        + SCALING * sum_e w[e] * (x @ lora_A[e].T) @ lora_B[e].T

Key identity used: with w folded into the rank-space activations,
    lora_out = (low * w_rep) @ B_all.T,  low = x @ A_all.T   (A_all: [E*R, D])
so the whole MoE-LoRA is two dense matmuls + tiny gating vector math.

Sharding: 8-way over tokens (512 tokens/core, full 4096 out features).
Token-only sharding means the LoRA-A + gate matmuls are not replicated
across out-feature groups, halving that tensor-engine work vs a 4x2 split.

All matmul operands are bf16 (same 1 cycle/row PE rate as f32r, same
effective precision, half the HBM traffic); PSUM accumulation is fp32.

Layout per core (everything "transposed", contraction dim on partitions):
    out.T[o, t] = sum_d W[o, d] * x.T[d, t]    (x.T moving, W tiles stationary)
"""

import numpy as np
import ml_dtypes

import concourse.bass as bass
import concourse.bass_isa as bass_isa
import concourse.mybir as mybir
import concourse.tile as tile
from concourse import bacc
from concourse.bass_utils import run_bass_kernel_spmd

F32 = mybir.dt.float32
BF16 = mybir.dt.bfloat16
NP_BF16 = ml_dtypes.bfloat16

# Problem constants
B, S, D, O = 2, 2048, 4096, 4096
E, R = 8, 16
ER = E * R  # 128
SCALING = 32.0 / 16.0

# Sharding: 8 token groups, full out-feature range per core
N_CORES = 8
TG = 8
T = (B * S) // TG       # 512 tokens per core
TO = O                  # 4096 out features per core
KT = D // 128           # 32 contraction tiles
OTN = TO // 128         # 32 out tiles per core
XC = 4                  # x DMA chunk: 4 k-tiles (512 KiB bf16)


def build_body(nc, tc, tensors):
    xT, wT, aT, gT, bT, bias2, Rm, out = tensors
    OP = mybir.AluOpType

    with (
        tc.tile_pool(name="xp", bufs=KT // XC) as xp,
        tc.tile_pool(name="wp", bufs=6) as wp,
        tc.tile_pool(name="cst", bufs=1) as cst,
        tc.tile_pool(name="gw", bufs=1) as gw,
        tc.tile_pool(name="outp", bufs=3) as outp,
        tc.tile_pool(name="psA", bufs=1, space="PSUM") as psA,
        tc.tile_pool(name="psG", bufs=2, space="PSUM") as psG,
        tc.tile_pool(name="psB", bufs=5, space="PSUM") as psB,
    ):
        # ---- startup DMA schedule. Phase A is strictly HBM-paced, so the
        #      sync queue (one FIFO ring == explicit HBM priority) carries
        #      everything phase A consumes, chunked and interleaved in the
        #      exact order the k-loop needs it: g, then (a, x) chunks, then
        #      the first base-W tiles. ----
        g_sb = cst.tile([128, KT, E], BF16)
        nc.sync.dma_start(out=g_sb[:], in_=gT[:])

        x_chunks = {}
        w_tiles = {}

        def load_x(g, eng):
            xg = xp.tile([128, XC, T], BF16, tag="x", name=f"x{g}")
            eng.dma_start(out=xg[:], in_=xT[:, g * XC:(g + 1) * XC, :])
            x_chunks[g] = xg

        def load_w(ot):
            wq = wp.tile([128, KT, 128], BF16, tag="w", name=f"w{ot}")
            nc.sync.dma_start(out=wq[:], in_=wT[:, ot, :, :])
            w_tiles[ot] = wq

        # Everything rides ONE ring (sync) in exact need-order: the ring's
        # FIFO is the HBM priority list. A second early ring only splits the
        # shared SDMA ramp and delays the first bytes (~5us measured). W0
        # rides as two half-tiles woven into the x stream so kloop0's 8-k
        # blocks can interleave with the gate pass as x trickles in.
        w0 = wp.tile([128, KT, 128], BF16, tag="w", name="w0")
        w_tiles[0] = w0
        load_x(0, nc.sync)
        load_x(1, nc.sync)
        nc.sync.dma_start(out=w0[:, 0:16, :], in_=wT[:, 0, 0:16, :])
        load_x(2, nc.sync)
        load_x(3, nc.sync)
        load_x(4, nc.sync)
        load_x(5, nc.sync)
        nc.sync.dma_start(out=w0[:, 16:32, :], in_=wT[:, 0, 16:32, :])
        load_x(6, nc.sync)
        load_x(7, nc.sync)

        def xk(k):
            return x_chunks[k // XC][:, k % XC, :]

        a_sb = cst.tile([128, KT, ER], BF16)
        nc.sync.dma_start(out=a_sb[:], in_=aT[:])
        load_w(1)
        bT_sb = cst.tile([ER, TO], BF16)
        nc.sync.dma_start(out=bT_sb[:], in_=bT[:])
        for ot in range(2, 6):
            load_w(ot)
        Rm_sb = cst.tile([E, ER], BF16)
        nc.gpsimd.dma_start(out=Rm_sb[:], in_=Rm[:])
        bias_sb = cst.tile([128, OTN], F32)
        nc.gpsimd.dma_start(out=bias_sb[:], in_=bias2[:])

        # ---- gate pass interleaved with kloop0 in halves: the x-load tail
        #      (k>=16) hides under kloop0's first half instead of adding
        #      serially. Runs of same-PSUM-bank matmuls pipeline at full
        #      rate (per-k alternation between two banks cost ~50% extra). ----
        low_ps = psA.tile([ER, T], F32, tag="low")
        gate_ps = psG.tile([E, T], F32, tag="g", name="gate")

        def gate_pass(k0, k1):
            for k in range(k0, k1):
                nc.tensor.matmul(gate_ps[:], lhsT=g_sb[:, k, :], rhs=xk(k),
                                 start=(k == 0), stop=(k == KT - 1))

        # ---- phase B k-loop (accumulate W-tile^T @ x.T into a PSUM bank) ----
        def kloop(ot, k0=0, k1=KT, pb=None):
            if pb is None:
                pb = psB.tile([128, T], F32, tag="pb", name=f"pb{ot}")
            for k in range(k0, k1):
                nc.tensor.matmul(pb[:], lhsT=w_tiles[ot][:, k, :], rhs=xk(k),
                                 start=(k == 0), stop=False)
            return pb

        # ---- PE warm-up: the tensor queue is otherwise empty for ~5us while
        #      the DMA ring ramps, and the HAM clock-gate only reaches 2.4GHz
        #      after ~3.4us of sustained activity. Burn that dead time on
        #      dummy matmuls over scratch SBUF (results discarded) so the
        #      real stream starts warm instead of at 1.2GHz. ----
        warm_sb = cst.tile([128, 640], BF16)
        nc.vector.memset(warm_sb[:], 0.0)
        warm_ps = psB.tile([128, T], F32, tag="pb", name="warm")
        for _ in range(8):
            nc.tensor.matmul(warm_ps[:], lhsT=warm_sb[:, 0:128],
                             rhs=warm_sb[:, 128:640], start=True, stop=True)

        pbs = {}
        for q in range(0, KT, 8):
            gate_pass(q, q + 8)
            pbs[0] = kloop(0, q, q + 8, pb=pbs.get(0))

        # ---- gating math in [E, t] layout (vector/scalar/gpsimd queues;
        #      runs concurrently with the tensor k-loops) ----
        g_sbf = gw.tile([E, T], F32, tag="gsb")
        nc.vector.tensor_copy(g_sbf[:], gate_ps[:])
        m1b = gw.tile([E, T], F32, tag="m1b")
        nc.gpsimd.partition_all_reduce(m1b[:], g_sbf[:], channels=E,
                                       reduce_op=bass_isa.ReduceOp.max)
        eq = gw.tile([E, T], F32, tag="tmp", bufs=3, name="eq")
        nc.vector.tensor_tensor(eq[:], g_sbf[:], m1b[:], op=OP.is_equal)
        gm = gw.tile([E, T], F32, tag="tmp", bufs=3, name="gm")
        nc.vector.scalar_tensor_tensor(gm[:], in0=eq[:], scalar=-1e30, in1=g_sbf[:],
                                       op0=OP.mult, op1=OP.add)
        m2b = gw.tile([E, T], F32, tag="m2b")
        nc.gpsimd.partition_all_reduce(m2b[:], gm[:], channels=E,
                                       reduce_op=bass_isa.ReduceOp.max)
        diff = gw.tile([E, T], F32, tag="tmp", bufs=3, name="diff")
        nc.vector.tensor_sub(diff[:], g_sbf[:], m1b[:])
        ex = gw.tile([E, T], F32, tag="ex")
        nc.scalar.activation(ex[:], diff[:], mybir.ActivationFunctionType.Exp)
        mask = gw.tile([E, T], F32, tag="tmp", bufs=3, name="mask")
        nc.vector.tensor_tensor(mask[:], g_sbf[:], m2b[:], op=OP.is_ge)
        wn = gw.tile([E, T], F32, tag="wn")
        nc.vector.tensor_mul(wn[:], ex[:], mask[:])
        # denominator 1 + exp(m2 - m1), broadcast on all 8 rows
        dmb = gw.tile([E, T], F32, tag="tmp", bufs=3, name="dmb")
        nc.vector.tensor_sub(dmb[:], m2b[:], m1b[:])
        edb = gw.tile([E, T], F32, tag="edb")
        nc.scalar.activation(edb[:], dmb[:], mybir.ActivationFunctionType.Exp)
        denb = gw.tile([E, T], F32, tag="tmp", bufs=3, name="denb")
        nc.vector.tensor_scalar_add(denb[:], edb[:], 1.0)
        recb = gw.tile([E, T], F32, tag="recb")
        nc.vector.reciprocal(recb[:], denb[:])
        wsc = gw.tile([E, T], BF16, tag="wsc")
        nc.vector.scalar_tensor_tensor(wsc[:], in0=wn[:], scalar=SCALING, in1=recb[:],
                                       op0=OP.mult, op1=OP.mult)

        # ---- LoRA-A pass (a arrives on the ring just after the x tail) ----
        for k in range(KT):
            nc.tensor.matmul(low_ps[:], lhsT=a_sb[:, k, :], rhs=xk(k),
                             start=(k == 0), stop=(k == KT - 1))

        pbs[1] = kloop(1)
        pbs[2] = kloop(2)

        # wrep: replicate each expert weight over its 16 ranks via tiny
        # matmul. Placed three k-loops after the gate pass finished: the
        # ~15us gating chain (cross-engine hops dominate) must be done by
        # the time the PE reaches this, else the PE idles AND goes cold.
        wrep_ps = psG.tile([ER, T], F32, tag="g", name="wrep")
        nc.tensor.matmul(wrep_ps[:], lhsT=Rm_sb[:], rhs=wsc[:],
                         start=True, stop=True)
        # low_w.T = low.T * w_rep  (copy wrep to SBUF first: DVE has a single
        # PSUM read port, two-PSUM-operand tensor_tensor is illegal)
        wrep_sb = gw.tile([ER, T], F32, tag="wrepsb")
        nc.scalar.copy(wrep_sb[:], wrep_ps[:])
        lowT_sb = gw.tile([ER, T], BF16, tag="lowT")
        nc.vector.tensor_tensor(lowT_sb[:], low_ps[:], wrep_sb[:], op=OP.mult)

        for ot in range(OTN):
            nxt = ot + 3
            if nxt < OTN:
                pbs[nxt] = kloop(nxt)
                if nxt + 3 < OTN:
                    load_w(nxt + 3)
            pb = pbs.pop(ot)
            nc.tensor.matmul(pb[:], lhsT=bT_sb[:, ot * 128:(ot + 1) * 128],
                             rhs=lowT_sb[:], start=False, stop=True)
            o_sb = outp.tile([128, T], F32, tag="o", name=f"o{ot}")
            # the last three B-matmuls drain back-to-back (no k-loops left
            # to interleave), so spread their bias+store across engines/rings
            # instead of serializing on vector + the gpsimd ring
            if ot == OTN - 2:
                nc.scalar.activation(o_sb[:], pb[:],
                                     mybir.ActivationFunctionType.Identity,
                                     bias=bias_sb[:, ot:ot + 1])
            else:
                nc.vector.tensor_scalar(o_sb[:], pb[:],
                                        scalar1=bias_sb[:, ot:ot + 1],
                                        scalar2=None, op0=OP.add)
            if ot == OTN - 1:
                # warm ring with an empty queue at this point
                nc.sync.dma_start(out=out[:, ot, :], in_=o_sb[:])
            else:
                nc.gpsimd.dma_start(out=out[:, ot, :], in_=o_sb[:])


def build_module(debug=False):
    nc = bacc.Bacc("TRN2", target_bir_lowering=False, debug=debug)
    xT = nc.dram_tensor("xT", [128, KT, T], BF16, kind="ExternalInput")
    wT = nc.dram_tensor("wT", [128, OTN, KT, 128], BF16, kind="ExternalInput")
    aT = nc.dram_tensor("aT", [128, KT, ER], BF16, kind="ExternalInput")
    gT = nc.dram_tensor("gT", [128, KT, E], BF16, kind="ExternalInput")
    bT = nc.dram_tensor("bT", [ER, TO], BF16, kind="ExternalInput")
    bias2 = nc.dram_tensor("bias2", [128, OTN], F32, kind="ExternalInput")
    Rm = nc.dram_tensor("Rm", [E, ER], BF16, kind="ExternalInput")
    out = nc.dram_tensor("out", [128, OTN, T], F32, kind="ExternalOutput")
    with tile.TileContext(nc) as tc:
        build_body(nc, tc, (xT, wT, aT, gT, bT, bias2, Rm, out))
    nc.compile()
    return nc


def shard_inputs(x, gate_w, base_w, base_b, lora_A, lora_B):
    """FULL inputs -> list of 8 per-core input maps (host-side, free)."""
    x = np.asarray(x, dtype=np.float32)
    gate_w = np.asarray(gate_w, dtype=np.float32)
    base_w = np.asarray(base_w, dtype=np.float32)
    base_b = np.asarray(base_b, dtype=np.float32)
    lora_A = np.asarray(lora_A, dtype=np.float32)
    lora_B = np.asarray(lora_B, dtype=np.float32)

    xf = x.reshape(B * S, D)
    # replicated smalls
    gT = np.ascontiguousarray(
        gate_w.T.reshape(KT, 128, E).transpose(1, 0, 2)).astype(NP_BF16)
    A_flat = lora_A.reshape(ER, D)
    aT = np.ascontiguousarray(
        A_flat.T.reshape(KT, 128, ER).transpose(1, 0, 2)).astype(NP_BF16)
    B_flat = lora_B.transpose(0, 2, 1).reshape(ER, O)   # [er, o]
    bT = np.ascontiguousarray(B_flat).astype(NP_BF16)
    Rm = np.repeat(np.eye(E, dtype=np.float32), R, axis=1).astype(NP_BF16)
    # replicated full base weight, bf16, contraction-on-partitions layout
    wT = np.ascontiguousarray(
        base_w.reshape(OTN, 128, KT, 128).transpose(3, 0, 2, 1)).astype(NP_BF16)
    bias2 = np.ascontiguousarray(base_b.reshape(OTN, 128).T)

    in_maps = []
    for c in range(N_CORES):
        x_c = xf[c * T:(c + 1) * T]                     # [T, D]
        xT = np.ascontiguousarray(
            x_c.T.reshape(KT, 128, T).transpose(1, 0, 2)).astype(NP_BF16)
        in_maps.append({"xT": xT, "wT": wT, "aT": aT, "gT": gT,
                        "bT": bT, "bias2": bias2, "Rm": Rm})
    return in_maps


def gather_outputs(results):
    """list of 8 per-core result maps -> FULL output [B, S, O]."""
    full = np.empty((B * S, O), dtype=np.float32)
    for c in range(N_CORES):
        oc = results[c]["out"]                          # [128, OTN, T]
        full[c * T:(c + 1) * T, :] = oc.transpose(2, 1, 0).reshape(T, O)
    return full.reshape(B, S, O)


_NC_CACHE = {}


def _get_module():
    if "nc" not in _NC_CACHE:
        _NC_CACHE["nc"] = build_module()
    return _NC_CACHE["nc"]


def run_sharded(in_maps, **run_kwargs):
    nc = _get_module()
    return run_bass_kernel_spmd(nc, in_maps, list(range(N_CORES)), **run_kwargs)


def kernel(x, gate_w, base_w, base_b, lora_A, lora_B):
    in_maps = shard_inputs(x, gate_w, base_w, base_b, lora_A, lora_B)
    res = run_sharded(in_maps)
    return gather_outputs(res.results)


# revision 32
# speedup vs baseline: 1.0269x; 1.0269x over previous
"""Trainium2 Bass kernel for a LoRA-MoE layer (gate top-2 softmax routing +
dense base linear + per-expert low-rank adapters), SPMD across 8 NeuronCores.

Math (per token t):
    logits = x @ gate_w.T                      # [E]
    top-2 softmax over logits -> dense w[E] (0 for non-selected)
    out = x @ base_w.T + base_b
        + SCALING * sum_e w[e] * (x @ lora_A[e].T) @ lora_B[e].T

Key identity used: with w folded into the rank-space activations,
    lora_out = (low * w_rep) @ B_all.T,  low = x @ A_all.T   (A_all: [E*R, D])
so the whole MoE-LoRA is two dense matmuls + tiny gating vector math.

Sharding: 8-way over tokens (512 tokens/core, full 4096 out features).
Token-only sharding means the LoRA-A + gate matmuls are not replicated
across out-feature groups, halving that tensor-engine work vs a 4x2 split.

All matmul operands are bf16 (same 1 cycle/row PE rate as f32r, same
effective precision, half the HBM traffic); PSUM accumulation is fp32.

Layout per core (everything "transposed", contraction dim on partitions):
    out.T[o, t] = sum_d W[o, d] * x.T[d, t]    (x.T moving, W tiles stationary)
"""

import numpy as np
import ml_dtypes

import concourse.bass as bass
import concourse.bass_isa as bass_isa
import concourse.mybir as mybir
import concourse.tile as tile
from concourse import bacc
from concourse.bass_utils import run_bass_kernel_spmd

F32 = mybir.dt.float32
BF16 = mybir.dt.bfloat16
NP_BF16 = ml_dtypes.bfloat16

# Problem constants
B, S, D, O = 2, 2048, 4096, 4096
E, R = 8, 16
ER = E * R  # 128
SCALING = 32.0 / 16.0

# Sharding: 8 token groups, full out-feature range per core
N_CORES = 8
TG = 8
T = (B * S) // TG       # 512 tokens per core
TO = O                  # 4096 out features per core
KT = D // 128           # 32 contraction tiles
OTN = TO // 128         # 32 out tiles per core
XC = 4                  # x DMA chunk: 4 k-tiles (512 KiB bf16)


def build_body(nc, tc, tensors):
    xT, wT, aT, gT, bT, bias2, Rm, out = tensors
    OP = mybir.AluOpType

    with (
        tc.tile_pool(name="xp", bufs=KT // XC) as xp,
        tc.tile_pool(name="wp", bufs=6) as wp,
        tc.tile_pool(name="cst", bufs=1) as cst,
        tc.tile_pool(name="gw", bufs=1) as gw,
        tc.tile_pool(name="outp", bufs=6) as outp,
        tc.tile_pool(name="psA", bufs=1, space="PSUM") as psA,
        tc.tile_pool(name="psG", bufs=2, space="PSUM") as psG,
        tc.tile_pool(name="psB", bufs=5, space="PSUM") as psB,
    ):
        # ---- startup DMA schedule. Phase A is strictly HBM-paced, so the
        #      sync queue (one FIFO ring == explicit HBM priority) carries
        #      everything phase A consumes, chunked and interleaved in the
        #      exact order the k-loop needs it: g, then (a, x) chunks, then
        #      the first base-W tiles. ----
        g_sb = cst.tile([128, KT, E], BF16)
        nc.sync.dma_start(out=g_sb[:], in_=gT[:])

        x_chunks = {}
        w_tiles = {}

        def load_x(g, eng):
            xg = xp.tile([128, XC, T], BF16, tag="x", name=f"x{g}")
            eng.dma_start(out=xg[:], in_=xT[:, g * XC:(g + 1) * XC, :])
            x_chunks[g] = xg

        def load_w(ot):
            wq = wp.tile([128, KT, 128], BF16, tag="w", name=f"w{ot}")
            nc.sync.dma_start(out=wq[:], in_=wT[:, ot, :, :])
            w_tiles[ot] = wq

        # Everything rides ONE ring (sync) in exact need-order: the ring's
        # FIFO is the HBM priority list. A second early ring only splits the
        # shared SDMA ramp and delays the first bytes (~5us measured). W0
        # rides as two half-tiles woven into the x stream so kloop0's 8-k
        # blocks can interleave with the gate pass as x trickles in.
        w0 = wp.tile([128, KT, 128], BF16, tag="w", name="w0")
        w_tiles[0] = w0
        load_x(0, nc.sync)
        load_x(1, nc.sync)
        nc.sync.dma_start(out=w0[:, 0:16, :], in_=wT[:, 0, 0:16, :])
        load_x(2, nc.sync)
        load_x(3, nc.sync)
        load_x(4, nc.sync)
        load_x(5, nc.sync)
        nc.sync.dma_start(out=w0[:, 16:32, :], in_=wT[:, 0, 16:32, :])
        load_x(6, nc.sync)
        load_x(7, nc.sync)

        def xk(k):
            return x_chunks[k // XC][:, k % XC, :]

        a_sb = cst.tile([128, KT, ER], BF16)
        nc.sync.dma_start(out=a_sb[:], in_=aT[:])
        load_w(1)
        bT_sb = cst.tile([ER, TO], BF16)
        nc.sync.dma_start(out=bT_sb[:], in_=bT[:])
        for ot in range(2, 6):
            load_w(ot)
        Rm_sb = cst.tile([E, ER], BF16)
        nc.gpsimd.dma_start(out=Rm_sb[:], in_=Rm[:])
        bias_sb = cst.tile([128, OTN], F32)
        nc.gpsimd.dma_start(out=bias_sb[:], in_=bias2[:])

        # ---- gate pass interleaved with kloop0 in halves: the x-load tail
        #      (k>=16) hides under kloop0's first half instead of adding
        #      serially. Runs of same-PSUM-bank matmuls pipeline at full
        #      rate (per-k alternation between two banks cost ~50% extra). ----
        low_ps = psA.tile([ER, T], F32, tag="low")
        gate_ps = psG.tile([E, T], F32, tag="g", name="gate")

        def gate_pass(k0, k1):
            for k in range(k0, k1):
                nc.tensor.matmul(gate_ps[:], lhsT=g_sb[:, k, :], rhs=xk(k),
                                 start=(k == 0), stop=(k == KT - 1))

        # ---- phase B k-loop (accumulate W-tile^T @ x.T into a PSUM bank) ----
        def kloop(ot, k0=0, k1=KT, pb=None):
            if pb is None:
                pb = psB.tile([128, T], F32, tag="pb", name=f"pb{ot}")
            for k in range(k0, k1):
                nc.tensor.matmul(pb[:], lhsT=w_tiles[ot][:, k, :], rhs=xk(k),
                                 start=(k == 0), stop=False)
            return pb

        # ---- PE warm-up: the tensor queue is otherwise empty for ~5us while
        #      the DMA ring ramps, and the HAM clock-gate only reaches 2.4GHz
        #      after ~3.4us of sustained activity. Burn that dead time on
        #      dummy matmuls over scratch SBUF (results discarded) so the
        #      real stream starts warm instead of at 1.2GHz. ----
        warm_sb = cst.tile([128, 640], BF16)
        nc.vector.memset(warm_sb[:], 0.0)
        warm_ps = psB.tile([128, T], F32, tag="pb", name="warm")
        for _ in range(8):
            nc.tensor.matmul(warm_ps[:], lhsT=warm_sb[:, 0:128],
                             rhs=warm_sb[:, 128:640], start=True, stop=True)

        pbs = {}
        for q in range(0, KT, 8):
            gate_pass(q, q + 8)
            pbs[0] = kloop(0, q, q + 8, pb=pbs.get(0))

        # ---- gating math in [E, t] layout (vector/scalar/gpsimd queues;
        #      runs concurrently with the tensor k-loops) ----
        g_sbf = gw.tile([E, T], F32, tag="gsb")
        nc.vector.tensor_copy(g_sbf[:], gate_ps[:])
        m1b = gw.tile([E, T], F32, tag="m1b")
        nc.gpsimd.partition_all_reduce(m1b[:], g_sbf[:], channels=E,
                                       reduce_op=bass_isa.ReduceOp.max)
        eq = gw.tile([E, T], F32, tag="tmp", bufs=3, name="eq")
        nc.vector.tensor_tensor(eq[:], g_sbf[:], m1b[:], op=OP.is_equal)
        gm = gw.tile([E, T], F32, tag="tmp", bufs=3, name="gm")
        nc.vector.scalar_tensor_tensor(gm[:], in0=eq[:], scalar=-1e30, in1=g_sbf[:],
                                       op0=OP.mult, op1=OP.add)
        m2b = gw.tile([E, T], F32, tag="m2b")
        nc.gpsimd.partition_all_reduce(m2b[:], gm[:], channels=E,
                                       reduce_op=bass_isa.ReduceOp.max)
        diff = gw.tile([E, T], F32, tag="tmp", bufs=3, name="diff")
        nc.vector.tensor_sub(diff[:], g_sbf[:], m1b[:])
        ex = gw.tile([E, T], F32, tag="ex")
        nc.scalar.activation(ex[:], diff[:], mybir.ActivationFunctionType.Exp)
        mask = gw.tile([E, T], F32, tag="tmp", bufs=3, name="mask")
        nc.vector.tensor_tensor(mask[:], g_sbf[:], m2b[:], op=OP.is_ge)
        wn = gw.tile([E, T], F32, tag="wn")
        nc.vector.tensor_mul(wn[:], ex[:], mask[:])
        # denominator 1 + exp(m2 - m1), broadcast on all 8 rows
        dmb = gw.tile([E, T], F32, tag="tmp", bufs=3, name="dmb")
        nc.vector.tensor_sub(dmb[:], m2b[:], m1b[:])
        edb = gw.tile([E, T], F32, tag="edb")
        nc.scalar.activation(edb[:], dmb[:], mybir.ActivationFunctionType.Exp)
        denb = gw.tile([E, T], F32, tag="tmp", bufs=3, name="denb")
        nc.vector.tensor_scalar_add(denb[:], edb[:], 1.0)
        recb = gw.tile([E, T], F32, tag="recb")
        nc.vector.reciprocal(recb[:], denb[:])
        wsc = gw.tile([E, T], BF16, tag="wsc")
        nc.vector.scalar_tensor_tensor(wsc[:], in0=wn[:], scalar=SCALING, in1=recb[:],
                                       op0=OP.mult, op1=OP.mult)

        # ---- LoRA-A pass (a arrives on the ring just after the x tail) ----
        for k in range(KT):
            nc.tensor.matmul(low_ps[:], lhsT=a_sb[:, k, :], rhs=xk(k),
                             start=(k == 0), stop=(k == KT - 1))

        pbs[1] = kloop(1)
        pbs[2] = kloop(2)

        # wrep: replicate each expert weight over its 16 ranks via tiny
        # matmul. Placed three k-loops after the gate pass finished: the
        # ~15us gating chain (cross-engine hops dominate) must be done by
        # the time the PE reaches this, else the PE idles AND goes cold.
        wrep_ps = psG.tile([ER, T], F32, tag="g", name="wrep")
        nc.tensor.matmul(wrep_ps[:], lhsT=Rm_sb[:], rhs=wsc[:],
                         start=True, stop=True)
        # low_w.T = low.T * w_rep  (copy wrep to SBUF first: DVE has a single
        # PSUM read port, two-PSUM-operand tensor_tensor is illegal)
        wrep_sb = gw.tile([ER, T], F32, tag="wrepsb")
        nc.scalar.copy(wrep_sb[:], wrep_ps[:])
        lowT_sb = gw.tile([ER, T], BF16, tag="lowT")
        nc.vector.tensor_tensor(lowT_sb[:], low_ps[:], wrep_sb[:], op=OP.mult)

        for ot in range(OTN):
            nxt = ot + 3
            if nxt < OTN:
                pbs[nxt] = kloop(nxt)
                if nxt + 3 < OTN:
                    load_w(nxt + 3)
            pb = pbs.pop(ot)
            nc.tensor.matmul(pb[:], lhsT=bT_sb[:, ot * 128:(ot + 1) * 128],
                             rhs=lowT_sb[:], start=False, stop=True)
            o_sb = outp.tile([128, T], F32, tag="o", name=f"o{ot}")
            # the last three B-matmuls drain back-to-back (no k-loops left
            # to interleave), so spread their bias+store across engines/rings
            # instead of serializing on vector + the gpsimd ring
            if ot == OTN - 2:
                nc.scalar.activation(o_sb[:], pb[:],
                                     mybir.ActivationFunctionType.Identity,
                                     bias=bias_sb[:, ot:ot + 1])
            else:
                nc.vector.tensor_scalar(o_sb[:], pb[:],
                                        scalar1=bias_sb[:, ot:ot + 1],
                                        scalar2=None, op0=OP.add)
            if ot == OTN - 1:
                # warm ring with an empty queue at this point
                nc.sync.dma_start(out=out[:, ot, :], in_=o_sb[:])
            else:
                nc.gpsimd.dma_start(out=out[:, ot, :], in_=o_sb[:])


def build_module(debug=False):
    nc = bacc.Bacc("TRN2", target_bir_lowering=False, debug=debug)
    xT = nc.dram_tensor("xT", [128, KT, T], BF16, kind="ExternalInput")
    wT = nc.dram_tensor("wT", [128, OTN, KT, 128], BF16, kind="ExternalInput")
    aT = nc.dram_tensor("aT", [128, KT, ER], BF16, kind="ExternalInput")
    gT = nc.dram_tensor("gT", [128, KT, E], BF16, kind="ExternalInput")
    bT = nc.dram_tensor("bT", [ER, TO], BF16, kind="ExternalInput")
    bias2 = nc.dram_tensor("bias2", [128, OTN], F32, kind="ExternalInput")
    Rm = nc.dram_tensor("Rm", [E, ER], BF16, kind="ExternalInput")
    out = nc.dram_tensor("out", [128, OTN, T], F32, kind="ExternalOutput")
    with tile.TileContext(nc) as tc:
        build_body(nc, tc, (xT, wT, aT, gT, bT, bias2, Rm, out))
    nc.compile()
    return nc


def shard_inputs(x, gate_w, base_w, base_b, lora_A, lora_B):
    """FULL inputs -> list of 8 per-core input maps (host-side, free)."""
    x = np.asarray(x, dtype=np.float32)
    gate_w = np.asarray(gate_w, dtype=np.float32)
    base_w = np.asarray(base_w, dtype=np.float32)
    base_b = np.asarray(base_b, dtype=np.float32)
    lora_A = np.asarray(lora_A, dtype=np.float32)
    lora_B = np.asarray(lora_B, dtype=np.float32)

    xf = x.reshape(B * S, D)
    # replicated smalls
    gT = np.ascontiguousarray(
        gate_w.T.reshape(KT, 128, E).transpose(1, 0, 2)).astype(NP_BF16)
    A_flat = lora_A.reshape(ER, D)
    aT = np.ascontiguousarray(
        A_flat.T.reshape(KT, 128, ER).transpose(1, 0, 2)).astype(NP_BF16)
    B_flat = lora_B.transpose(0, 2, 1).reshape(ER, O)   # [er, o]
    bT = np.ascontiguousarray(B_flat).astype(NP_BF16)
    Rm = np.repeat(np.eye(E, dtype=np.float32), R, axis=1).astype(NP_BF16)
    # replicated full base weight, bf16, contraction-on-partitions layout
    wT = np.ascontiguousarray(
        base_w.reshape(OTN, 128, KT, 128).transpose(3, 0, 2, 1)).astype(NP_BF16)
    bias2 = np.ascontiguousarray(base_b.reshape(OTN, 128).T)

    in_maps = []
    for c in range(N_CORES):
        x_c = xf[c * T:(c + 1) * T]                     # [T, D]
        xT = np.ascontiguousarray(
            x_c.T.reshape(KT, 128, T).transpose(1, 0, 2)).astype(NP_BF16)
        in_maps.append({"xT": xT, "wT": wT, "aT": aT, "gT": gT,
                        "bT": bT, "bias2": bias2, "Rm": Rm})
    return in_maps


def gather_outputs(results):
    """list of 8 per-core result maps -> FULL output [B, S, O]."""
    full = np.empty((B * S, O), dtype=np.float32)
    for c in range(N_CORES):
        oc = results[c]["out"]                          # [128, OTN, T]
        full[c * T:(c + 1) * T, :] = oc.transpose(2, 1, 0).reshape(T, O)
    return full.reshape(B, S, O)


_NC_CACHE = {}


def _get_module():
    if "nc" not in _NC_CACHE:
        _NC_CACHE["nc"] = build_module()
    return _NC_CACHE["nc"]


def run_sharded(in_maps, **run_kwargs):
    nc = _get_module()
    return run_bass_kernel_spmd(nc, in_maps, list(range(N_CORES)), **run_kwargs)


def kernel(x, gate_w, base_w, base_b, lora_A, lora_B):
    in_maps = shard_inputs(x, gate_w, base_w, base_b, lora_A, lora_B)
    res = run_sharded(in_maps)
    return gather_outputs(res.results)
